# revision 1
# baseline (speedup 1.0000x reference)
"""YOLOv7 batch assigner (dense-masked cross-grid assignment) on 8 Trainium2 cores.

The reference only reads the pred tensors' static shapes (80/40/20 feature maps)
— never their values — so the kernel touches none of that data. The real work
operates on batch_targets_normed (3,1024,7) + tiny priors/grid-offset constants
and produces (3, 15360, 6).

Sharding: the 1024 GTs are split 128-per-core across 8 cores; 128 GTs map
exactly onto the 128 SBUF partitions. Priors/offsets are replicated to every
core inside a single fused (128, 148) input tile (together with shape-derived
scale-factor tables).

Per-core inputs are two tensors so the first DMA (everything the opening ops
need) can land while the second is still in flight; replication across
levels/anchors/offsets happens via 0-stride APs on-device. Combo k = 3*level
+ anchor:
  inp1 (128, 24):  [0:21) T: this core's targets, col 7a+f;
                   [21:24) SF3: per-level scale W_i (levels are square, W==H;
                   cols 0,1,6 of the scaled block are never read, so one
                   scalar per level suffices)
  inp2 (128, 64):  [0:18) PB, col 2k+c; [18:28) OFF, col 18+2o+c;
                   [28:46) WHC (W,H), col 28+2k+c; [46:64) W1 (W-1,H-1), col 46+2k+c

Output per core is three tensors (m = (level*5 + offset)*3 + anchor), each
DMA'd out as soon as its two field columns are computed so transfers overlap
the remaining compute:
  out1  (128, 90): col 2m+c = [pw, ph]
  out2a (128, 90): col 2m+c = [img, prior]
  out2b (128, 90): col 2m+c = [gx, gy]
The host transposes/stitches them into (3, 15360, 6).

All floor/frac math uses the exact round-to-nearest magic (v+2^23)-2^23 with an
is_gt correction — bit-identical to the reference's floor/mod for v >= 0.
"""

from contextlib import contextmanager

import numpy as np

import concourse.bass as bass
import concourse.mybir as mybir
from concourse import bass_utils

f32 = mybir.dt.float32
Alu = mybir.AluOpType
Axis = mybir.AxisListType

N_CORES = 8
A = 3
G = 1024
GL = G // N_CORES  # 128 GTs per core == SBUF partitions
FEATS = [(80, 80), (40, 40), (20, 20)]
THR = 4.0
NEAR = 0.5
MAGIC = 8388608.0  # 2**23: (v + MAGIC) - MAGIC == round-to-nearest-even(v), |v| < 2**22
IN1_COLS = 24
IN2_COLS = 64
OUT_COLS = 90  # each output tensor: two interleaved field columns


def _ap(base: bass.AP, col: int, dims: list[list[int]]) -> bass.AP:
    """AP addressing columns of a (128, N) SBUF tile: partition dim + custom free dims."""
    sl = base[:, col : col + 1]
    return bass.AP(tensor=sl.tensor, offset=sl.offset, ap=[sl.ap[0]] + dims)


def _ap_range(ap: bass.AP) -> tuple[str, int, int]:
    """(tensor_name, lo, hi) span of an AP's free-dim footprint (conservative)."""
    lo = ap.offset
    span = 1
    for step, count in ap.ap[1:]:
        span += abs(step) * (count - 1)
    return ap.tensor.name, lo, lo + span


class _Chain:
    """Emit ops on one engine with semaphore waits for same-engine RAW hazards.

    DVE reads sample SBUF early in the pipe while writes retire late, so an op
    reading a prior op's output needs a sem wait (bare back-to-back issue gave
    corrupted results on HW). WAR/WAW are safe in issue order. mode:
      "full" — wait before every op (what CoreSim's race detector verifies)
      "raw"  — wait only when an input overlaps a previously written range
    Every op increments the sem so SP can gate the output DMA on the total."""

    def __init__(self, eng, sem, mode="raw"):
        self._eng = eng
        self._sem = sem
        self._mode = mode
        self.n = 0
        self._waited = 0
        self._writes: list[tuple[str, int, int, int]] = []  # (tensor, lo, hi, idx)

    def _emit(self, name, *a, **k):
        aps = [x for x in a if isinstance(x, bass.AP)]
        out, ins = aps[0], aps[1:]
        if self._mode == "full":
            need = self.n
        else:
            need = 0
            for ap in ins:
                t, lo, hi = _ap_range(ap)
                for wt, wlo, whi, idx in self._writes:
                    if wt == t and lo < whi and wlo < hi:
                        need = max(need, idx)
        if need > self._waited:
            self._eng.wait_ge(self._sem, need)
            self._waited = need
        inst = getattr(self._eng, name)(*a, **k)
        inst.then_inc(self._sem, 1)
        self.n += 1
        t, lo, hi = _ap_range(out)
        self._writes.append((t, lo, hi, self.n))
        return inst

    def __getattr__(self, name):
        return lambda *a, **k: self._emit(name, *a, **k)


def _emit_compute(nc: bass.Bass, inp1: bass.AP, inp2: bass.AP, outt1: bass.AP,
                  outt2a: bass.AP, outt2b: bass.AP, tl, v, dma_sem=None) -> dict:
    """All compute ops (DVE program order). tl(name, cols) allocates an SBUF tile
    AP. Returns chain marks gating the three output DMAs.

    If dma_sem is given, a wait for the second input DMA is emitted before the
    first inp2 consumer."""
    s = tl("s", 63)
    c4t = tl("c4t", 36)
    match = tl("match", 9)
    vd, vr, dd = tl("vd", 36), tl("vr", 36), tl("dd", 36)
    d3 = tl("d3", 108)
    maskp, mask = tl("maskp", 45), tl("mask", 45)
    xya, xym = tl("xya", 90), tl("xym", 90)
    xyc, fr, fc, fn = tl("xyc", 108), tl("fr", 108), tl("fc", 108), tl("fn", 108)

    # Op order interleaves the two independent chains (match/dirs vs coords) so
    # that almost every dependent hop has >=2 unrelated ops between producer
    # and consumer — the consumer's RAW wait is then already satisfied instead
    # of stalling on the producer's pipeline retire.
    pb_ck = _ap(inp2, 0, [[1, 2], [2, 9]])
    g29 = [[9, 2], [1, 9]]
    o59 = [[9, 5], [1, 9]]
    s_wh = _ap(s, 4, [[1, 2], [7, 9]])
    s_xy = _ap(s, 2, [[1, 2], [7, 9]])

    # S = T * W_level (scaled targets; T bcast over levels, one scale per level
    # since W==H; s cols {0,1,6} are scaled too but never read)
    v.tensor_tensor(_ap(s, 0, [[21, 3], [1, 21]]),
                    _ap(inp1, 0, [[0, 3], [1, 21]]),
                    _ap(inp1, 21, [[1, 3], [0, 21]]), Alu.mult)
    v.memset(maskp[:, 0:9], 1.0)
    # append img/prior (unscaled, read straight from T) to the floor batch
    v.tensor_copy(_ap(xyc, 90, [[6, 3], [2, 3], [1, 2]]),
                  _ap(inp1, 0, [[0, 3], [7, 3], [6, 2]]))

    if dma_sem is not None:
        nc.vector.wait_ge(dma_sem, 16)

    # match = (w < 4p) & (p < 4w) & (h < 4q) & (q < 4h): 2 pair-batched compares,
    # AND = group-min. c4t groups: [4pw>w | 4ph>h | 4w>pw | 4h>ph]
    v.scalar_tensor_tensor(_ap(c4t, 0, g29), pb_ck, THR, s_wh, Alu.mult, Alu.is_gt)
    v.scalar_tensor_tensor(_ap(c4t, 18, g29), s_wh, THR, pb_ck, Alu.mult, Alu.is_gt)

    # direction flags for v in [x | y | W-x | H-y] (one 36-col batch):
    #   (mod(v,1) < 0.5) & (v > 1)  ==  (d >= 0) & (d < 0.5) & (v > 1),
    # where d = v - rne(v) (exact; ties resolve identically to the reference)
    v.tensor_copy(_ap(vd, 0, g29), s_xy)
    v.tensor_sub(_ap(vd, 18, g29), _ap(inp2, 28, [[1, 2], [2, 9]]), s_xy)

    # adjusted grid coords, all 5 offsets at once: xa[o,k] = x[k] - off_x[o]*0.5
    # (two ops: ScalarTensorTensor APs are limited to partition + 2 free dims)
    co_k = [[45, 2], [9, 5], [1, 9]]
    v.scalar_tensor_tensor(_ap(xya, 0, o59), _ap(inp2, 18, [[2, 5], [0, 9]]), -NEAR,
                           _ap(s, 2, [[0, 5], [7, 9]]), Alu.mult, Alu.add)
    v.scalar_tensor_tensor(_ap(xya, 45, o59), _ap(inp2, 19, [[2, 5], [0, 9]]), -NEAR,
                           _ap(s, 3, [[0, 5], [7, 9]]), Alu.mult, Alu.add)

    v.tensor_reduce(match[:], _ap(c4t, 0, [[1, 9], [9, 4]]), Axis.X, Alu.min)
    v.tensor_scalar(vr[:], vd[:], MAGIC, MAGIC, Alu.add, Alu.subtract)
    v.tensor_scalar(xym[:], xya[:], 0.0, None, Alu.max)      # clip lo
    v.tensor_sub(dd[:], vd[:], vr[:])
    v.tensor_tensor(_ap(xyc, 0, co_k), _ap(xym, 0, co_k),
                    _ap(inp2, 46, [[1, 2], [0, 5], [2, 9]]), Alu.min)  # clip hi (W-1|H-1)
    v.tensor_scalar(d3[:, 0:36], dd[:], 0.0, None, Alu.is_ge)
    v.tensor_scalar(d3[:, 36:72], dd[:], NEAR, None, Alu.is_lt)
    v.tensor_scalar(d3[:, 72:108], vd[:], 1.0, None, Alu.is_gt)

    # floor(v) = rne(v) - corr, corr = (rne(v) > v); exact for v >= 0.
    # (NOT the f32->int32 round-trip: HW converts with round-to-nearest even
    # though CoreSim truncates — HW-verified divergence.)
    # fn cols: [0:45) floor(gx), [45:90) floor(gy), [90:108) floor(img|pri)
    v.tensor_scalar(fr[:], xyc[:], MAGIC, MAGIC, Alu.add, Alu.subtract)
    v.tensor_reduce(maskp[:, 9:45], _ap(d3, 0, [[1, 36], [36, 3]]), Axis.X, Alu.min)
    v.tensor_tensor(fc[:], fr[:], xyc[:], Alu.is_gt)

    # mask = dir & match (match broadcast over the 5 offsets)
    v.tensor_tensor(mask[:], maskp[:], _ap(match, 0, [[0, 5], [1, 9]]), Alu.mult)

    # assemble outputs; m = (i*5 + o)*3 + a, each tensor col 2m+c
    pwph = [[30, 3], [6, 5], [2, 3]]
    mask_oia = [[3, 3], [9, 5], [1, 3]]    # 45-col tiles indexed o*9 + 3i + a
    ip_ia = [[6, 3], [0, 5], [2, 3]]       # 18-col (k,c) pairs, broadcast over o
    mpos = _ap(mask, 0, mask_oia)
    v.tensor_tensor(_ap(outt1, 0, pwph), _ap(inp2, 0, ip_ia), mpos, Alu.mult)
    v.tensor_tensor(_ap(outt1, 1, pwph), _ap(inp2, 1, ip_ia), mpos, Alu.mult)
    marks = {"ph": v.n}
    v.tensor_sub(fn[:], fr[:], fc[:])
    v.tensor_tensor(_ap(outt2a, 0, pwph), _ap(fn, 90, ip_ia), mpos, Alu.mult)
    v.tensor_tensor(_ap(outt2a, 1, pwph), _ap(fn, 91, ip_ia), mpos, Alu.mult)
    marks["ip"] = v.n
    v.tensor_tensor(_ap(outt2b, 0, pwph), _ap(fn, 0, mask_oia), mpos, Alu.mult)
    v.tensor_tensor(_ap(outt2b, 1, pwph), _ap(fn, 45, mask_oia), mpos, Alu.mult)
    return marks


class _NoBarrierBlock(bass.BassBlock):
    """BassBlock without the exit-time all-engine drain+barrier.

    Single-block kernel: each engine's stream quiesces at its own end and SP
    already waits for the output DMA, so the inter-engine barrier is pure tail
    overhead."""

    def __exit__(self, exc_type, exc_val, exc_tb):
        if exc_type is not None:
            return
        for engine, last_body in self.last_body.items():
            with self.bass.body(
                last_body, parent=self.bass.cur_bb, allow_existing_parent=True
            ):
                engine.br(self.end_bb)
        self.bass.switch_bb(self.end_bb)


@contextmanager
def _no_barrier_block(nc):
    assert nc.cur_block is None
    blk = _NoBarrierBlock(nc, f"block_{nc.next_id()}")
    with blk:
        nc.cur_block = blk
        yield blk
    nc.cur_block = None


def _build_nc(reps: int = 1, mode: str = "raw", barrier: bool = False) -> bass.Bass:
    """Raw Bass (no TileContext): linear pipeline DMA-in -> DVE ops -> DMA-out.

    Manual sync is two semaphores; no kernel-tail drain/barrier.
    reps>1 replicates the compute body (for marginal-time measurement only).
    mode="full" chains every op (for CoreSim's race detector)."""
    nc = bass.Bass("TRN2", debug=False)
    inp1_d = nc.dram_tensor("inp1", (GL, IN1_COLS), f32, kind="ExternalInput").ap()
    inp2_d = nc.dram_tensor("inp2", (GL, IN2_COLS), f32, kind="ExternalInput").ap()
    out1_d = nc.dram_tensor("out1", (GL, OUT_COLS), f32, kind="ExternalOutput").ap()
    out2a_d = nc.dram_tensor("out2a", (GL, OUT_COLS), f32, kind="ExternalOutput").ap()
    out2b_d = nc.dram_tensor("out2b", (GL, OUT_COLS), f32, kind="ExternalOutput").ap()

    tiles = {}

    def tl(name, cols, dtype=f32):
        if name not in tiles:
            tiles[name] = nc.alloc_sbuf_tensor(name, [GL, cols], dtype).ap()
        return tiles[name]

    inp1 = tl("inp1_sb", IN1_COLS)
    inp2 = tl("inp2_sb", IN2_COLS)
    outt1 = tl("out1_sb", OUT_COLS)
    outt2a = tl("out2a_sb", OUT_COLS)
    outt2b = tl("out2b_sb", OUT_COLS)

    blk_ctx = nc.Block() if barrier else _no_barrier_block(nc)
    with (
        nc.semaphore("dma_in1") as dma_in1,
        nc.semaphore("dma_in2") as dma_in2,
        nc.semaphore("dma_out") as dma_out,
        nc.semaphore("vchain") as vchain,
        blk_ctx as block,
    ):
        n_ops = {}

        @block.vector
        def _(vector):
            vector.wait_ge(dma_in1, 16)
            ch = _Chain(nc.vector, vchain, mode=mode)
            for _r in range(reps):
                marks = _emit_compute(nc, inp1, inp2, outt1, outt2a, outt2b, tl, ch,
                                      dma_sem=dma_in2 if _r == 0 else None)
            n_ops["n"] = ch.n
            n_ops["ph"] = marks["ph"] if reps == 1 else ch.n
            n_ops["ip"] = marks["ip"] if reps == 1 else ch.n

        @block.sync
        def _(sync):
            sync.dma_start(inp1[:], inp1_d[:]).then_inc(dma_in1, 16)
            sync.dma_start(inp2[:], inp2_d[:]).then_inc(dma_in2, 16)
            sync.wait_ge(vchain, n_ops["ph"])
            sync.dma_start(out1_d[:], outt1[:]).then_inc(dma_out, 16)
            sync.wait_ge(vchain, n_ops["ip"])
            sync.dma_start(out2a_d[:], outt2a[:]).then_inc(dma_out, 16)
            sync.wait_ge(vchain, n_ops["n"])
            sync.dma_start(out2b_d[:], outt2b[:]).then_inc(dma_out, 16)
            sync.wait_ge(dma_out, 48)

    return nc


_NC_CACHE: bass.Bass | None = None


def _get_nc() -> bass.Bass:
    global _NC_CACHE
    if _NC_CACHE is None:
        _NC_CACHE = _build_nc()
    return _NC_CACHE


def _host_inputs(batch_targets_normed, priors_base_sizes, grid_offset):
    tgt = np.asarray(batch_targets_normed, dtype=np.float32)  # (3, 1024, 7)
    pbs = np.asarray(priors_base_sizes, dtype=np.float32)      # (3, 3, 2)
    goff = np.asarray(grid_offset, dtype=np.float32)           # (5, 1, 2)

    pb = np.broadcast_to(pbs.reshape(1, 18), (GL, 18))
    off = np.broadcast_to(goff.reshape(1, 10), (GL, 10))
    whc = np.empty((9, 2), np.float32)
    sf = np.empty((3,), np.float32)
    w1 = np.empty((9, 2), np.float32)
    for i, (h_, w_) in enumerate(FEATS):
        sf[i] = w_
        for a in range(A):
            whc[i * 3 + a] = (w_, h_)
            w1[i * 3 + a] = (w_ - 1.0, h_ - 1.0)
    whc = np.broadcast_to(whc.reshape(1, 18), (GL, 18))
    sf = np.broadcast_to(sf.reshape(1, 3), (GL, 3))
    w1 = np.broadcast_to(w1.reshape(1, 18), (GL, 18))
    inp2 = np.ascontiguousarray(np.concatenate([pb, off, whc, w1], axis=1))  # (128, 64)

    in_maps = []
    for c in range(N_CORES):
        t_c = tgt[:, c * GL : (c + 1) * GL, :].transpose(1, 0, 2).reshape(GL, 21)
        inp1 = np.ascontiguousarray(np.concatenate([t_c, sf], axis=1))       # (128, 42)
        in_maps.append({"inp1": inp1, "inp2": inp2})
    return in_maps


def _gather(results) -> np.ndarray:
    full = np.empty((3, 5, A, N_CORES, GL, 6), np.float32)
    for c in range(N_CORES):
        # each out tensor: (gt, m, 2 fields); m = (i,o,a)
        for name, f0 in (("out2a", 0), ("out2b", 2), ("out1", 4)):
            o = np.asarray(results[c][name]).reshape(GL, 3, 5, A, 2)
            full[:, :, :, c, :, f0 : f0 + 2] = o.transpose(1, 2, 3, 0, 4)
    return np.ascontiguousarray(full.reshape(3, 5 * A * G, 6))


def kernel(pred0, pred1, pred2, batch_targets_normed, priors_base_sizes,
           grid_offset, batch_input_shape, _profile_kwargs=None):
    in_maps = _host_inputs(batch_targets_normed, priors_base_sizes, grid_offset)
    nc = _get_nc()
    res = bass_utils.run_bass_kernel_spmd(
        nc, in_maps, core_ids=list(range(N_CORES)), **(_profile_kwargs or {})
    )
    out = _gather(res.results)
    if _profile_kwargs:
        return out, res
    return out

